# revision 20
# baseline (speedup 1.0000x reference)
"""Trainium2 Bass kernel for nn_Baseline_635655160228 (retrieval_knn).

Reference computation (B=64, WAYS=10, SHOTS=5, C=128, H=W=32):
    cov_j = centered-Gram(support_j) / (N-1)          # [ways, C, C], N = shots*hw
    qn    = q / ||q||_2(per channel row)              # [B, C, hw]
    sim[b,j,p] = qn_p^T cov_j qn_p                    # diag quadratic form
    out[b,j]   = sum_p leaky_relu(sim) * conv_w[p]

Key algebraic restructuring:
  cov_j is PSD (Gram of centered data), hence sim >= 0 and LeakyReLU is the
  identity.  Then
      out[b,j] = sum_p w_p qn_p^T cov_j qn_p = <cov_j, W_b>_F
  with W_b = qn diag(w) qn^T a tiny [C,C] matrix per query.

Distribution over 8 NeuronCores:
  - data-parallel over the query batch (8 queries per core)
  - covariance Grams sharded over the sample axis: each core takes a
    contiguous 640-sample slice (of shots*hw = 5120) of every way, handed to
    the device pre-transposed to [ways, C, 640] so the input DMA is fully
    contiguous.  Partial Grams + row sums are combined with one in-kernel
    bf16 AllReduce.
  - the AllReduce staging DMA (cc_in) is pushed on the gpsimd software DGE
    queue and the bulk q loads are held back until it is in flight, so the
    collective triggers as soon as the partial Grams exist instead of
    tailing megabytes of input descriptors.
  - mean correction applied at the end:
      out[b,j] = <R_j, W_b> - (1/N) m_j^T W_b m_j     (R raw Gram, m row sums)
    with 1/(N-1) folded into conv_w.

All on-chip transposes (support chunks and normalized-query chunks) are done
by the DMA transpose XBAR (16x128 tiles, bf16) instead of PE matmul
transposes: the Gram and W_b accumulations sum over all samples/pixels, so
the XBAR's row->(partition, chunk) assignment is just a relabeling; only
conv_w's on-chip layout has to match it.

All bulk matmul operands are bf16 (fp32 matmul runs at 1/4 rate on the PE
array); accumulation stays fp32 in PSUM.
"""

import numpy as np

B, WAYS, SHOTS, C, H, W = 64, 10, 5, 128, 32, 32
HW = H * W                       # 1024
NCORES = 8
BLOC = B // NCORES               # 8 queries per core
NTOT = SHOTS * HW                # 5120 samples per way
NLOC = NTOT // NCORES            # 640 samples per way per core
DENOM = float(NTOT - 1)          # 5119
SCH = NLOC // 128                # 5 transposed sample-chunks per way
QCH = HW // 128                  # 8 pixel chunks per query

_CACHE = {}


def _build_program():
    import concourse.bass as bass
    import concourse.tile as tile
    from concourse import bacc, mybir

    f32 = mybir.dt.float32
    bf16 = mybir.dt.bfloat16
    AF = mybir.ActivationFunctionType
    ALU = mybir.AluOpType

    nc = bacc.Bacc("TRN2", target_bir_lowering=False, debug=False,
                   num_devices=NCORES)

    q_d = nc.dram_tensor("q", [BLOC, C, HW], f32, kind="ExternalInput")
    sup_d = nc.dram_tensor("support", [WAYS, C, NLOC], f32,
                           kind="ExternalInput")
    w_d = nc.dram_tensor("conv_w", [HW], f32, kind="ExternalInput")
    out_d = nc.dram_tensor("out", [WAYS, BLOC], f32, kind="ExternalOutput")

    # collective bounce buffers: [C, ways*C Gram partials + ways row sums]
    CCW = WAYS * C + WAYS
    cc_in = nc.dram_tensor("cc_in", [C, CCW], bf16)
    cc_out = nc.dram_tensor("cc_out", [C, CCW], bf16,
                            addr_space="Shared")

    groups = [list(range(NCORES))]

    with tile.TileContext(nc) as tc:
        with (
            tc.tile_pool(name="const", bufs=1) as constp,
            tc.tile_pool(name="big", bufs=1) as big,
            tc.tile_pool(name="scratch", bufs=2) as scratch,
            tc.tile_pool(name="tp_ps", bufs=2, space="PSUM") as tp_ps,
            tc.tile_pool(name="gram_ps", bufs=2, space="PSUM") as gram_ps,
            tc.tile_pool(name="w_ps", bufs=2, space="PSUM") as w_ps,
            tc.tile_pool(name="fr_ps", bufs=1, space="PSUM") as fr_ps,
        ):
            import ml_dtypes
            ident_d = nc.inline_tensor(
                np.eye(128, dtype=ml_dtypes.bfloat16), name="ident_const")
            ident = constp.tile([128, 128], bf16, tag="ident")

            # selection matrix summing the col-group partial scores:
            # SEL[32u + j, j] = 1  (3 col groups — quadrant 3 has a HW bug)
            sel_np = np.zeros((128, WAYS), np.float32)
            for u in range(3):
                for j in range(WAYS):
                    sel_np[32 * u + j, j] = 1.0
            sel_d = nc.inline_tensor(sel_np, name="sel_const")
            sel = constp.tile([128, WAYS], f32, tag="sel")

            wp = constp.tile([128, QCH], f32, tag="wp")        # conv_w
            wps = constp.tile([128, QCH], f32, tag="wps")      # conv_w/(N-1)

            # ---------------- persistent tensors ----------------
            sup_nat = big.tile([C, WAYS, NLOC], f32, tag="sup_nat")
            sup_bf = big.tile([C, WAYS, NLOC], bf16, tag="sup_bf")
            xts = big.tile([128, WAYS, SCH, C], bf16, tag="xts")
            rsb = big.tile([C, CCW], bf16, tag="rsb")
            rall = big.tile([C, CCW], bf16, tag="rall")
            msum = constp.tile([C, WAYS], f32, tag="msum")
            qnat = big.tile([C, BLOC, HW], f32, tag="qnat")
            qbf = big.tile([C, BLOC, HW], bf16, tag="qbf")
            qT = big.tile([128, BLOC, QCH, C], bf16, tag="qT")
            wqT = big.tile([128, BLOC, QCH, C], bf16, tag="wqT")
            wsb = big.tile([C, BLOC, C], bf16, tag="wsb")

            nsq = constp.tile([128, BLOC], f32, tag="nsq")
            rin = constp.tile([128, BLOC], f32, tag="rin")
            tnw = constp.tile([128, BLOC], f32, tag="tnw")
            mallN = constp.tile([C, WAYS], bf16, tag="mallN")
            msT = constp.tile([WAYS, C], f32, tag="msT")
            ytmp = constp.tile([WAYS, BLOC, C], f32, tag="ytmp")
            ysb = constp.tile([WAYS, BLOC], f32, tag="ysb")
            fin = constp.tile([WAYS, BLOC], f32, tag="fin")

            # ---------------- input DMAs: support first ----------------
            # support is pre-transposed host-side to [ways, C, 640] so each
            # way is one clean 2.5KB-row DMA.  sync/scalar HWDGE queues.
            nc.gpsimd.dma_start(ident[:], ident_d[:])
            sup_dmas = []
            for j in range(WAYS):
                eng = nc.sync if j % 2 == 0 else nc.scalar
                sup_dmas.append(eng.dma_start(sup_nat[:, j, :], sup_d[j]))
            nc.sync.dma_start(wp[:], w_d.rearrange("(i p) -> p i", p=128))
            nc.gpsimd.dma_start(sel[:], sel_d[:])
            nc.vector.tensor_scalar_mul(wps[:], wp[:], 1.0 / DENOM)

            # PE warm-up while support lands: dummy matmuls release the HAM
            # clock gate (cold PE runs at 1.2 GHz, warm at 2.4 GHz)
            warm = fr_ps.tile([128, 128], f32, tag="score")
            last_warm = None
            for wi in range(24):
                last_warm = nc.tensor.matmul(
                    warm[:], lhsT=ident[:], rhs=ident[:],
                    start=(wi == 0), stop=(wi == 23))

            # ---------------- stage S: local support Grams ----------------
            # cast f32->bf16 on DVE; transpose each way's 5 [C,128] chunks on
            # the PE (any sample relabeling is fine for a Gram); accumulate
            # 5 chunk matmuls per way into PSUM.  Row sums m_j accumulate on
            # the idle ACT engine in f32.
            last_gram = None
            last_rcopy = None
            for j in range(WAYS):
                nc.vector.tensor_copy(sup_bf[:, j, :], sup_nat[:, j, :])
                pt = tp_ps.tile([128, SCH, 128], bf16, tag="tp")
                for t in range(SCH):
                    t_ = nc.tensor.transpose(
                        pt[:, t, :], sup_bf[:, j, 128 * t:128 * (t + 1)],
                        ident[:])
                    if j == 0 and t == 0:
                        tile.add_dep_helper(
                            t_.ins, last_warm.ins,
                            reason="PE warm-up before stage S")
                nc.vector.tensor_copy(xts[:, j, :, :], pt[:])
                msc = scratch.tile([C, NLOC], f32, tag="msc")
                nc.scalar.activation(msc[:], sup_nat[:, j, :], AF.Copy,
                                     accum_out=msum[:, j:j + 1])
                gp = gram_ps.tile([C, C], f32, tag="gram")
                for t in range(SCH):
                    g_ = nc.tensor.matmul(
                        gp[:], lhsT=xts[:, j, t, :],
                        rhs=xts[:, j, t, :],
                        start=(t == 0), stop=(t == SCH - 1))
                last_gram = g_
                nc.vector.tensor_copy(rsb[:, j * C:(j + 1) * C], gp[:])
            last_rcopy = nc.vector.tensor_copy(rsb[:, WAYS * C:], msum[:])

            # ---------------- AllReduce of Gram partials (bf16) -------------
            # cc_in staging goes on the gpsimd software DGE queue (empty at
            # this point); the bulk q loads below are held until it is
            # pushed so it never tails input descriptors.
            cc_push = nc.gpsimd.dma_start(cc_in[:], rsb[:])
            nc.gpsimd.collective_compute(
                "AllReduce", ALU.add, replica_groups=groups,
                ins=[cc_in[:]], outs=[cc_out[:]],
            )

            # ---------------- q loads (after cc_in is in flight) ------------
            q_engs = [nc.sync, nc.scalar] * (BLOC // 2)
            for b in range(BLOC):
                qd = q_engs[b].dma_start(qnat[:, b, :], q_d[b])
                tile.add_dep_helper(
                    qd.ins, cc_push.ins,
                    reason="hold q bytes until cc_in is in flight")

            # split the collective result load across 3 queues
            for e, eng in enumerate([nc.sync, nc.scalar, nc.gpsimd]):
                c0, c1 = (CCW * e) // 3, (CCW * (e + 1)) // 3
                eng.dma_start(rall[:, c0:c1], cc_out[:, c0:c1])
            rallR = rall[:, 0:WAYS * C].rearrange("c (j d) -> c j d", d=C)
            rallM = rall[:, WAYS * C:]

            # ---------------- stage Q: query norms + transposes -------------
            for b in range(BLOC):
                sq = scratch.tile([C, HW], f32, tag="sq")
                nc.scalar.activation(sq[:], qnat[:, b, :], AF.Square,
                                     accum_out=nsq[:, b:b + 1])
            # rinv = nsq^(-1/2) by Newton from constant seed (nsq ~ 1024)
            # (DVE stage-Q work explicitly ordered after stage-S's last copy
            # so the scheduler cannot convoy stage S behind the query chain)
            r0 = 2.0 ** -5
            first_nw = nc.vector.tensor_scalar(tnw[:], nsq[:],
                                               r0 * r0 * -0.5, 1.5,
                                               ALU.mult, ALU.add)
            tile.add_dep_helper(first_nw.ins, last_rcopy.ins,
                                reason="stage-S DVE before stage-Q DVE")
            nc.vector.tensor_scalar_mul(rin[:], tnw[:], r0)
            for _ in range(2):
                nc.vector.tensor_mul(tnw[:], rin[:], rin[:])
                nc.vector.tensor_mul(tnw[:], tnw[:], nsq[:])
                nc.vector.tensor_scalar(tnw[:], tnw[:], -0.5, 1.5,
                                        ALU.mult, ALU.add)
                nc.vector.tensor_mul(rin[:], rin[:], tnw[:])
            # qn = q * rinv (cast to bf16), transpose via XBAR, then scale
            # the transposed copy by conv_w/(N-1) (per-partition, DVE)
            first_wmm = None
            for b in range(BLOC):
                nc.vector.tensor_scalar_mul(qbf[:, b, :], qnat[:, b, :],
                                            rin[:, b:b + 1])
                for g in range(2):
                    pt = tp_ps.tile([128, SCH, 128], bf16, tag="tp")
                    for i in range(4):
                        ci = 4 * g + i
                        t_ = nc.tensor.transpose(
                            pt[:, i, :],
                            qbf[:, b, 128 * ci:128 * (ci + 1)], ident[:])
                        if first_wmm is None:
                            first_wmm = t_
                            tile.add_dep_helper(
                                t_.ins, last_gram.ins,
                                reason="stage-S PE before stage-Q PE")
                    nc.vector.tensor_copy(qT[:, b, 4 * g:4 * g + 4, :],
                                          pt[:, 0:4, :])
                nc.vector.tensor_tensor(
                    wqT[:, b, :, :], qT[:, b, :, :],
                    wps[:, :, None].to_broadcast((128, QCH, C)),
                    ALU.mult)
                # ---------------- stage W: W_b = (w' qn) qn^T ----------------
                wpt = w_ps.tile([C, C], f32, tag="wacc")
                for i in range(QCH):
                    nc.tensor.matmul(wpt[:], lhsT=wqT[:, b, i, :],
                                     rhs=qT[:, b, i, :],
                                     start=(i == 0), stop=(i == QCH - 1))
                nc.vector.tensor_copy(wsb[:, b, :], wpt[:])

            # ---------------- mean-correction prep ----------------
            # mallN = -m/N  (m = all-reduced row sums) ; msT = m^T
            nc.scalar.activation(mallN[:], rallM, AF.Copy,
                                 scale=-1.0 / NTOT)
            mt = w_ps.tile([WAYS, C], f32, tag="wacc")
            nc.tensor.matmul(mt[:], lhsT=rallM, rhs=ident[:],
                             start=True, stop=True)
            nc.vector.tensor_copy(msT[:], mt[:])

            # ---------------- correction: -(1/N) m^T W_b m ----------------
            # u[j,(b,d)] = sum_c (-m[j,c]/N) W[b,c,d] ; y = sum_d u * m[j,d]
            for h in range(2):
                up = w_ps.tile([WAYS, BLOC * C // 2], f32, tag="wacc")
                nc.tensor.matmul(up[:], lhsT=mallN[:],
                                 rhs=wsb[:, 4 * h:4 * (h + 1), :],
                                 start=True, stop=True)
                nc.vector.tensor_tensor(
                    ytmp[:, 4 * h:4 * (h + 1), :],
                    up[:].rearrange("j (b d) -> j b d", d=C),
                    msT[:, None, :].to_broadcast((WAYS, BLOC // 2, C)),
                    ALU.mult)
            nc.vector.tensor_reduce(ysb[:], ytmp[:],
                                    axis=mybir.AxisListType.X, op=ALU.add)

            # ---------------- Frobenius: score[j,b] = <R_j, W_b> -------------
            # 3 concurrent accumulations in PE column groups 0-2; col group
            # u handles c0 = 3k+u, partial scores land at partitions 32u+j.
            score4 = fr_ps.tile([128, BLOC], f32, tag="score")
            nc.vector.memset(score4[:], 0.0)
            for c0 in range(C):
                u = c0 % 3
                nc.tensor.matmul(score4[32 * u:32 * u + WAYS, :],
                                 lhsT=rallR[:, :, c0], rhs=wsb[:, :, c0],
                                 tile_position=(0, 32 * u),
                                 start=(c0 == 0), stop=(c0 == C - 1),
                                 skip_group_check=(c0 != 0 and c0 != C - 1))
            scr_sb = constp.tile([128, BLOC], f32, tag="scr_sb")
            nc.vector.tensor_copy(scr_sb[:], score4[:])
            fin_ps = w_ps.tile([WAYS, BLOC], f32, tag="wacc")
            nc.tensor.matmul(fin_ps[:], lhsT=sel[:], rhs=scr_sb[:],
                             start=True, stop=True)

            nc.vector.tensor_add(fin[:], fin_ps[:], ysb[:])
            nc.sync.dma_start(out_d[:], fin[:])

    nc.compile()
    return nc


def _get_program():
    if "nc" not in _CACHE:
        _CACHE["nc"] = _build_program()
    return _CACHE["nc"]


def _make_in_maps(q, support, conv_w):
    q = np.ascontiguousarray(np.asarray(q, dtype=np.float32)).reshape(B, C, HW)
    # [ways, shots, C, h, w] -> [ways, C, shots*hw]  (sample axis last)
    sup = np.asarray(support, dtype=np.float32).reshape(
        WAYS, SHOTS, C, HW).transpose(0, 2, 1, 3).reshape(WAYS, C, NTOT)
    w = np.ascontiguousarray(np.asarray(conv_w, dtype=np.float32))
    in_maps = []
    for k in range(NCORES):
        in_maps.append({
            "q": np.ascontiguousarray(q[k * BLOC:(k + 1) * BLOC]),
            "support": np.ascontiguousarray(
                sup[:, :, k * NLOC:(k + 1) * NLOC]),
            "conv_w": w,
        })
    return in_maps


def _run(in_maps, trace=False):
    from concourse.bass_utils import run_bass_kernel_spmd
    nc = _get_program()
    return run_bass_kernel_spmd(nc, in_maps, list(range(NCORES)), trace=trace)


def kernel(q, support, conv_w):
    res = _run(_make_in_maps(q, support, conv_w))
    out = np.concatenate(
        [res.results[k]["out"].T for k in range(NCORES)], axis=0)
    return np.ascontiguousarray(out.astype(np.float32))
